# revision 1
# baseline (speedup 1.0000x reference)
"""GraphSAGE-max (3 layers + 2 heads) on 8 Trainium2 NeuronCores.

Strategy: data-parallel over dst-node partitions (the "graph partition +
replicated weights" scheme). Nodes are dealt to the 8 cores snake-wise by
in-degree, then re-sorted inside each core by (in-degree, lo-half-degree)
so a dense ELL gather schedule has little padding. Features live in
replicated DRAM tables of bf16 rows; the per-core table block carries its
own -inf pad row so both the lo half (cores 0-3) and the hi half (cores
4-7) of the table are addressable with int16 dma_gather indices.

Each layer, per 128-node tile:
  - dma_gather neighbor rows (two calls: lo table view, hi table view)
    -> [128, K*F] bf16, tree-max down to [128, F], upcast to f32
  - PE-transpose to feature-major, then f32 matmuls:
      yT = relu(Wl.T @ aggT + Wr.T @ hT + b)
  - PE-transpose back to node-major, cast bf16, store to the core's block
  - AllGather blocks across the 8 cores -> next layer's gather table
The two output heads share the third aggregation.
"""

import numpy as np
import ml_dtypes

import concourse.bass as bass
import concourse.bacc as bacc
import concourse.mybir as mybir
import concourse.tile as tile
from concourse.masks import make_identity
from concourse.bass_utils import run_bass_kernel_spmd

N = 50000
E = 800000
F_IN = 128
H = 256
NCOR = 8
NLOC = N // NCOR             # 6250
BLOCK = NLOC + 1             # 6251 rows per core block (last = -inf pad)
HALF = 4 * BLOCK             # 25004 rows per table half
TILES = (NLOC + 127) // 128  # 49
PADN = TILES * 128           # 6272
NEG = float(np.finfo(np.float32).min)
KCAP = 8                     # max gather columns per dma_gather call
CHUNK = 4                    # node tiles per matmul chunk (N free = 512)
PADIDX = NLOC                # pad row local index inside a table half

_LAST = {}                   # stash for the test harness


# ----------------------------------------------------------------------------
# host-side graph preprocessing
# ----------------------------------------------------------------------------

def _wrap_idx(ilist):
    """ilist [NCOR, num] int -> dma_gather wrapped layout [NCOR, 128*ceil(num/16)]
    (16-partition wrap, replicated to 128 partitions)."""
    num = ilist.shape[1]
    cols = (num + 15) // 16
    w = np.zeros((NCOR, 16, cols), np.int16)
    i = np.arange(num)
    w[:, i % 16, i // 16] = ilist
    w = np.tile(w, (1, 8, 1))                 # [NCOR, 128, cols]
    return w.reshape(NCOR, 128 * cols)


def _preprocess(edge_index):
    src = np.asarray(edge_index[0], np.int64)
    dst = np.asarray(edge_index[1], np.int64)
    deg = np.bincount(dst, minlength=N)

    # deal nodes (by degree desc) to cores snake-wise -> owner per old id
    order = np.argsort(-deg, kind="stable")
    ranks = np.arange(N)
    pos = ranks % NCOR
    core_of_rank = np.where((ranks // NCOR) % 2 == 0, pos, NCOR - 1 - pos)
    owner = np.empty(N, np.int64)
    owner[order] = core_of_rank

    # lo half = nodes owned by cores 0-3
    lo_of_old = owner < 4
    deg_lo = np.bincount(dst[lo_of_old[src]], minlength=N)

    # within-core order: (deg desc, deg_lo desc) -> tight two-phase ELL
    old_of_new = np.empty(N, np.int64)
    for m in range(NCOR):
        nodes = np.where(owner == m)[0]
        key = np.lexsort((-deg_lo[nodes], -deg[nodes]))
        old_of_new[m * NLOC:(m + 1) * NLOC] = nodes[key]
    new_of_old = np.empty(N, np.int64)
    new_of_old[old_of_new] = np.arange(N)

    # local index within the table half, per old id
    m_of_old = new_of_old // NLOC
    r_of_old = new_of_old % NLOC
    tloc_of_old = np.where(m_of_old < 4, m_of_old, m_of_old - 4) * BLOCK + r_of_old

    # per-dst phase-split neighbor slots
    nd = new_of_old[dst]
    ph = (~lo_of_old[src]).astype(np.int64)           # 0 = lo, 1 = hi
    stloc = tloc_of_old[src]
    gk = nd * 2 + ph
    eorder = np.argsort(gk, kind="stable")
    gk_s = gk[eorder]
    stloc_s = stloc[eorder]
    starts = np.searchsorted(gk_s, np.arange(2 * N))
    slot = np.arange(E) - starts[gk_s]
    cnt = np.bincount(gk, minlength=2 * N)
    dlo = cnt[0::2]                  # lo-degree, indexed by new id
    dhi = cnt[1::2]

    # shared compile-time K schedule per (tile, phase): max over cores
    def ktile(d):
        ks = np.zeros(TILES, np.int64)
        for m in range(NCOR):
            dm = d[m * NLOC:(m + 1) * NLOC]
            for t in range(TILES):
                blk = dm[t * 128:(t + 1) * 128]
                if blk.size:
                    ks[t] = max(ks[t], int(blk.max()))
        return np.maximum(ks, 1)
    klo = ktile(dlo)
    khi = ktile(dhi)

    # dense ELL per phase [NCOR, PADN, kmax]
    def ell_of(phase, kmax):
        ell = np.full((NCOR, PADN, kmax), PADIDX, np.int16)
        sel = ph[eorder] == phase
        nd_e = nd[eorder][sel]
        ell[nd_e // NLOC, nd_e % NLOC, slot[sel]] = stloc_s[sel].astype(np.int16)
        return ell
    ells = {0: ell_of(0, int(klo.max())), 1: ell_of(1, int(khi.max()))}

    # gather call schedule + wrapped int16 index stream.
    # Calls are grouped per matmul chunk (CHUNK node tiles): one idx DMA
    # loads the whole chunk's wrapped indices; each call slices columns.
    NCH = (TILES + CHUNK - 1) // CHUNK
    sched = []           # (tile, phase, col0, kn, chunk, cidx0_cols)
    chunks = []          # (flat_off, cols) per chunk
    blocks = []
    off = 0
    for c in range(NCH):
        cblocks = []
        ccols = 0
        for t in range(c * CHUNK, min((c + 1) * CHUNK, TILES)):
            col0 = 0
            for phase, ks in ((0, klo), (1, khi)):
                k0 = 0
                while k0 < int(ks[t]):
                    kn = min(KCAP, int(ks[t]) - k0)
                    blk = ells[phase][:, t * 128:(t + 1) * 128, k0:k0 + kn]
                    ilist = blk.transpose(0, 2, 1).reshape(NCOR, kn * 128)
                    w = _wrap_idx(ilist).reshape(NCOR, 128, 8 * kn)
                    cblocks.append(w)
                    sched.append((t, phase, col0, kn, c, ccols))
                    ccols += 8 * kn
                    k0 += kn
                    col0 += kn
        cb = np.concatenate(cblocks, axis=2)      # [NCOR, 128, ccols]
        blocks.append(cb.reshape(NCOR, 128 * ccols))
        chunks.append((off, ccols))
        off += 128 * ccols
    idx_flat = np.concatenate(blocks, axis=1)
    ktot = klo + khi

    return dict(new_of_old=new_of_old, old_of_new=old_of_new,
                sched=sched, chunks=chunks, totslot=off, idx_flat=idx_flat,
                ktot=ktot, isolated=bool((deg == 0).any()))


# ----------------------------------------------------------------------------
# device program
# ----------------------------------------------------------------------------

def _tree_max(nc, g, k, F):
    """In-place max over k column groups of width F; result in g[:, :F]."""
    while k > 1:
        if k % 2 == 1:
            nc.vector.tensor_tensor(out=g[:, 0:F], in0=g[:, 0:F],
                                    in1=g[:, (k - 1) * F:k * F],
                                    op=mybir.AluOpType.max)
            k -= 1
            if k == 1:
                break
        half = k // 2
        nc.vector.tensor_tensor(out=g[:, 0:half * F], in0=g[:, 0:half * F],
                                in1=g[:, half * F:2 * half * F],
                                op=mybir.AluOpType.max)
        k = half


def _build_program(sched, chunks, totslot, isolated, stages=5):
    """stages: 0=L1 gathers only, 1=L1 only, 2=+AG1, 3=+L2, 4=+AG2, 5=full."""
    nc = bacc.Bacc("TRN2", target_bir_lowering=False, debug=False,
                   num_devices=NCOR)
    f32, bf16, i16 = mybir.dt.float32, mybir.dt.bfloat16, mybir.dt.int16

    t_xtab = nc.dram_tensor("xtab", [2 * HALF, F_IN], bf16,
                            kind="ExternalInput")
    t_xT = nc.dram_tensor("xT", [F_IN, PADN], f32, kind="ExternalInput")
    t_idx = nc.dram_tensor("idx", [totslot], i16, kind="ExternalInput")
    wnames = ["Wl1", "Wr1", "Wl2", "Wr2", "Wla", "Wra", "Wlm", "Wrm"]
    wshapes = {"Wl1": (F_IN, H), "Wr1": (F_IN, H)}
    t_w = {w: nc.dram_tensor(w, list(wshapes.get(w, (H, H))), f32,
                             kind="ExternalInput") for w in wnames}
    t_b = {b: nc.dram_tensor(b, [H, 1], f32, kind="ExternalInput")
           for b in ["bl1", "bl2", "bla", "blm"]}
    t_wh = {w: nc.dram_tensor(w, [H, 1], f32, kind="ExternalInput")
            for w in ["Wa", "Wm"]}
    t_bh = {b: nc.dram_tensor(b, [1, 1], f32, kind="ExternalInput")
            for b in ["ba", "bm"]}
    t_out = nc.dram_tensor("out", [2, NLOC], f32, kind="ExternalOutput")

    NCH = (TILES + CHUNK - 1) // CHUNK
    cw_of = lambda c: min(CHUNK, TILES - c * CHUNK) * 128

    sched_of_tile = {}
    for (t, phase, col0, kn, c, cidx0) in sched:
        sched_of_tile.setdefault(t, []).append((phase, col0, kn, cidx0))
    CMAX = max(cols for (_, cols) in chunks)

    with tile.TileContext(nc) as tc:
        with tc.tile_pool(name="const", bufs=1) as cpool, \
             tc.tile_pool(name="hT", bufs=1) as hpool, \
             tc.tile_pool(name="work", bufs=2) as wk, \
             tc.tile_pool(name="psT", bufs=4, space="PSUM") as psT, \
             tc.tile_pool(name="psY", bufs=2, space="PSUM") as psY, \
             tc.tile_pool(name="dram", bufs=1, space="DRAM") as dram:

            ident = cpool.tile([128, 128], f32, name="ident")
            make_identity(nc, ident[:])

            w_sb = {}
            for w in wnames:
                fi = wshapes.get(w, (H, H))[0]
                fh = fi // 128
                ws = cpool.tile([128, fh * H], f32, name=f"sb_{w}")
                for h in range(fh):
                    nc.sync.dma_start(ws[:, h * H:(h + 1) * H],
                                      t_w[w][h * 128:(h + 1) * 128, :])
                w_sb[w] = ws
            b_sb = {}
            for b in t_b:
                bs = cpool.tile([128, 2], f32, name=f"sb_{b}")
                for h in range(2):
                    nc.sync.dma_start(bs[:, h:h + 1],
                                      t_b[b][h * 128:(h + 1) * 128, :])
                b_sb[b] = bs
            wh_sb = {}
            for w in t_wh:
                ws = cpool.tile([128, 2], f32, name=f"sb_{w}")
                for h in range(2):
                    nc.sync.dma_start(ws[:, h:h + 1],
                                      t_wh[w][h * 128:(h + 1) * 128, :])
                wh_sb[w] = ws
            bh_sb = {}
            for b in t_bh:
                bs = cpool.tile([1, 1], f32, name=f"sb_{b}")
                nc.sync.dma_start(bs[:], t_bh[b][:])
                bh_sb[b] = bs

            xT_sb = hpool.tile([128, PADN], f32, name="xT_sb")
            nc.sync.dma_start(xT_sb[:], t_xT[:])
            h1T = hpool.tile([128, 2 * PADN], f32, name="h1T")
            h2T = hpool.tile([128, 2 * PADN], f32, name="h2T")

            h1tab = dram.tile([2 * HALF, H], bf16, name="h1tab",
                              addr_space="Shared")
            h2tab = dram.tile([2 * HALF, H], bf16, name="h2tab",
                              addr_space="Shared")
            blk1 = dram.tile([BLOCK, H], bf16, name="blk1")
            blk2 = dram.tile([BLOCK, H], bf16, name="blk2")

            # each core's block ends with a -inf pad row
            padrow = cpool.tile([1, H], bf16, name="padrow")
            nc.vector.memset(padrow[:], NEG)
            nc.sync.dma_start(blk1[NLOC:NLOC + 1, :], padrow[:])
            nc.sync.dma_start(blk2[NLOC:NLOC + 1, :], padrow[:])

            def load_idx_chunk(c, tag):
                off, cols = chunks[c]
                idxc = wk.tile([128, CMAX], i16, name=f"idxc_{tag}",
                               tag="idxc", bufs=3)
                nc.sync.dma_start(
                    idxc[:, :cols],
                    t_idx[off:off + 128 * cols].rearrange("(p s) -> p s",
                                                          p=128))
                return idxc

            def aggregate_tile(t, table, F, tag, idxc):
                """two-phase gather + tree-max + upcast for node tile t.
                Returns an f32 [128, F] tile."""
                agg16 = wk.tile([128, H], bf16, name=f"agg16_{tag}",
                                tag="agg16")
                first = True
                for (phase, col0, kn, cidx0) in sched_of_tile[t]:
                    cols = 8 * kn
                    g = wk.tile([128, KCAP * H], bf16, name=f"g_{tag}",
                                tag="gather", bufs=3)
                    view = table[0:HALF, :] if phase == 0 \
                        else table[HALF:2 * HALF, :]
                    nc.gpsimd.dma_gather(
                        out_ap=g[:, :kn * F].rearrange("p (k f) -> p k f",
                                                       f=F),
                        in_ap=view, idxs_ap=idxc[:, cidx0:cidx0 + cols],
                        num_idxs=128 * kn, num_idxs_reg=128 * kn,
                        elem_size=F, single_packet=False)
                    _tree_max(nc, g, kn, F)
                    if first:
                        nc.vector.tensor_copy(agg16[:, :F], g[:, :F])
                        first = False
                    else:
                        nc.vector.tensor_tensor(out=agg16[:, :F],
                                                in0=agg16[:, :F],
                                                in1=g[:, :F],
                                                op=mybir.AluOpType.max)
                agg32 = wk.tile([128, H], f32, name=f"agg32_{tag}",
                                tag="agg32")
                nc.vector.tensor_copy(agg32[:, :F], agg16[:, :F])
                return agg32

            def transpose_into(srcap, dst, col, tag):
                tp = psT.tile([128, 128], f32, name=f"tp_{tag}", tag="tp")
                nc.tensor.transpose(tp[:], srcap, ident[:])
                nc.vector.tensor_copy(dst[:, col:col + 128], tp[:])

            def layer(table, selfT, F, Wl, Wr, bl, outT, blkout, tag):
                fh_in = F // 128
                if stages == 0:
                    for c in range(NCH):
                        idxc = load_idx_chunk(c, f"{tag}_{c}")
                        for i in range(cw_of(c) // 128):
                            t = c * CHUNK + i
                            agg32 = aggregate_tile(t, table, F, f"{tag}_{t}",
                                                   idxc)
                            rows = min(128, NLOC - t * 128)
                            nc.sync.dma_start(
                                blkout[t * 128:t * 128 + rows, 0:F],
                                agg32[:rows, :F])
                    return
                for c in range(NCH):
                    cw = cw_of(c)
                    ntile = cw // 128
                    idxc = load_idx_chunk(c, f"{tag}_{c}")
                    aggT = wk.tile([128, fh_in * 512], f32,
                                   name=f"aggT_{tag}", tag="aggT")
                    for i in range(ntile):
                        t = c * CHUNK + i
                        agg32 = aggregate_tile(t, table, F, f"{tag}_{t}",
                                               idxc)
                        for fh in range(fh_in):
                            transpose_into(agg32[:, fh * 128:(fh + 1) * 128],
                                           aggT, fh * 512 + i * 128,
                                           f"{tag}_{t}_{fh}")
                    for hh in range(2):
                        psy = psY.tile([128, 512], f32, name=f"psy_{tag}",
                                       tag="psy")
                        nmm = 2 * fh_in
                        i = 0
                        for fh in range(fh_in):
                            nc.tensor.matmul(
                                psy[:, :cw],
                                w_sb[Wl][:, fh * H + hh * 128:
                                         fh * H + (hh + 1) * 128],
                                aggT[:, fh * 512:fh * 512 + cw],
                                start=(i == 0), stop=(i == nmm - 1))
                            i += 1
                            nc.tensor.matmul(
                                psy[:, :cw],
                                w_sb[Wr][:, fh * H + hh * 128:
                                         fh * H + (hh + 1) * 128],
                                selfT[:, fh * PADN + c * CHUNK * 128:
                                      fh * PADN + c * CHUNK * 128 + cw],
                                start=(i == 0), stop=(i == nmm - 1))
                            i += 1
                        nc.scalar.activation(
                            outT[:, hh * PADN + c * CHUNK * 128:
                                 hh * PADN + c * CHUNK * 128 + cw],
                            psy[:, :cw],
                            mybir.ActivationFunctionType.Relu,
                            bias=b_sb[bl][:, hh:hh + 1])
                    for i in range(ntile):
                        t = c * CHUNK + i
                        ynode = wk.tile([128, H], bf16, name=f"yn_{tag}",
                                        tag="ynode")
                        for hh in range(2):
                            tp = psT.tile([128, 128], f32,
                                          name=f"tpo_{tag}", tag="tp")
                            nc.tensor.transpose(
                                tp[:],
                                outT[:, hh * PADN + t * 128:
                                     hh * PADN + (t + 1) * 128],
                                ident[:])
                            nc.vector.tensor_copy(
                                ynode[:, hh * 128:(hh + 1) * 128], tp[:])
                        rows = min(128, NLOC - t * 128)
                        nc.sync.dma_start(blkout[t * 128:t * 128 + rows, :],
                                          ynode[:rows, :])

            layer(t_xtab, xT_sb, F_IN, "Wl1", "Wr1", "bl1", h1T, blk1, "l1")
            if stages == 0:
                pass
            if stages >= 2:
                nc.gpsimd.collective_compute(
                    "AllGather", mybir.AluOpType.bypass,
                    replica_groups=[list(range(NCOR))],
                    ins=[blk1.opt()], outs=[h1tab.opt()])
            if stages >= 3:
                layer(h1tab, h1T, H, "Wl2", "Wr2", "bl2", h2T, blk2, "l2")
            if stages >= 4:
                nc.gpsimd.collective_compute(
                    "AllGather", mybir.AluOpType.bypass,
                    replica_groups=[list(range(NCOR))],
                    ins=[blk2.opt()], outs=[h2tab.opt()])

            # layer 3: two branches + heads
            for c in range(NCH if stages >= 5 else 0):
                cw = cw_of(c)
                ntile = cw // 128
                idxc = load_idx_chunk(c, f"l3_{c}")
                aggT = wk.tile([128, 2 * 512], f32, name="aggT_l3",
                               tag="aggT")
                for i in range(ntile):
                    t = c * CHUNK + i
                    agg32 = aggregate_tile(t, h2tab, H, f"l3_{t}", idxc)
                    for fh in range(2):
                        transpose_into(agg32[:, fh * 128:(fh + 1) * 128],
                                       aggT, fh * 512 + i * 128,
                                       f"l3_{t}_{fh}")
                out_sbs = [wk.tile([1, 512], f32, name=f"out_sb{bi}",
                                   tag=f"out_sb{bi}") for bi in range(2)]
                for bi, (Wl, Wr, bl, Wh, bh) in enumerate(
                        [("Wla", "Wra", "bla", "Wa", "ba"),
                         ("Wlm", "Wrm", "blm", "Wm", "bm")]):
                    brT = wk.tile([128, 2 * 512], f32, name=f"brT{bi}",
                                  tag="brT")
                    for hh in range(2):
                        psy = psY.tile([128, 512], f32, name=f"psy3_{bi}",
                                       tag="psy")
                        for fh in range(2):
                            nc.tensor.matmul(
                                psy[:, :cw],
                                w_sb[Wl][:, fh * H + hh * 128:
                                         fh * H + (hh + 1) * 128],
                                aggT[:, fh * 512:fh * 512 + cw],
                                start=(fh == 0), stop=False)
                            nc.tensor.matmul(
                                psy[:, :cw],
                                w_sb[Wr][:, fh * H + hh * 128:
                                         fh * H + (hh + 1) * 128],
                                h2T[:, fh * PADN + c * CHUNK * 128:
                                    fh * PADN + c * CHUNK * 128 + cw],
                                start=False, stop=(fh == 1))
                        nc.scalar.activation(
                            brT[:, hh * 512:hh * 512 + cw], psy[:, :cw],
                            mybir.ActivationFunctionType.Relu,
                            bias=b_sb[bl][:, hh:hh + 1])
                    psh = psY.tile([1, 512], f32, name=f"psh{bi}", tag="psh")
                    for hh in range(2):
                        nc.tensor.matmul(psh[:, :cw],
                                         wh_sb[Wh][:, hh:hh + 1],
                                         brT[:, hh * 512:hh * 512 + cw],
                                         start=(hh == 0), stop=(hh == 1))
                    nc.scalar.activation(out_sbs[bi][:, :cw],
                                         psh[:, :cw],
                                         mybir.ActivationFunctionType.Identity,
                                         bias=bh_sb[bh][:])
                live = min(cw, NLOC - c * CHUNK * 128)
                for bi in range(2):
                    nc.sync.dma_start(
                        t_out[bi:bi + 1,
                              c * CHUNK * 128:c * CHUNK * 128 + live],
                        out_sbs[bi][:, :live])

    nc.compile()
    return nc


# ----------------------------------------------------------------------------
# entry point
# ----------------------------------------------------------------------------

def kernel(x, edge_index, Wl1, bl1, Wr1, Wl2, bl2, Wr2,
           Wla, bla, Wra, Wa, ba, Wlm, blm, Wrm, Wm, bm):
    x = np.asarray(x, np.float32)
    pp = _preprocess(edge_index)
    old_of_new = pp["old_of_new"]

    # x gather table in block layout: per core 6250 rows + one -inf pad row
    xp = x[old_of_new]
    xtab = np.empty((2 * HALF, F_IN), np.float32)
    for m in range(NCOR):
        base = m * BLOCK if m < 4 else HALF + (m - 4) * BLOCK
        xtab[base:base + NLOC] = xp[m * NLOC:(m + 1) * NLOC]
        xtab[base + NLOC] = NEG
    xtab = xtab.astype(ml_dtypes.bfloat16)

    nc = _build_program(pp["sched"], pp["chunks"], pp["totslot"],
                        pp["isolated"])

    def f32(a):
        return np.ascontiguousarray(np.asarray(a, np.float32))

    in_maps = []
    for m in range(NCOR):
        blk = xp[m * NLOC:(m + 1) * NLOC]
        xT = np.zeros((F_IN, PADN), np.float32)
        xT[:, :NLOC] = blk.T
        in_maps.append({
            "xtab": xtab, "xT": xT, "idx": pp["idx_flat"][m],
            "Wl1": f32(Wl1), "Wr1": f32(Wr1),
            "Wl2": f32(Wl2), "Wr2": f32(Wr2),
            "Wla": f32(Wla), "Wra": f32(Wra),
            "Wlm": f32(Wlm), "Wrm": f32(Wrm),
            "bl1": f32(bl1).reshape(H, 1), "bl2": f32(bl2).reshape(H, 1),
            "bla": f32(bla).reshape(H, 1), "blm": f32(blm).reshape(H, 1),
            "Wa": f32(Wa).reshape(H, 1), "Wm": f32(Wm).reshape(H, 1),
            "ba": f32(ba).reshape(1, 1), "bm": f32(bm).reshape(1, 1),
        })

    res = run_bass_kernel_spmd(nc, in_maps, core_ids=list(range(NCOR)))

    rt = np.empty(N, np.float32)
    mv = np.empty(N, np.float32)
    for m in range(NCOR):
        out = res.results[m]["out"]
        rt[m * NLOC:(m + 1) * NLOC] = out[0]
        mv[m * NLOC:(m + 1) * NLOC] = out[1]
    rt_o = np.empty(N, np.float32)
    mv_o = np.empty(N, np.float32)
    rt_o[old_of_new] = rt
    mv_o[old_of_new] = mv

    _LAST.update(nc=nc, in_maps=in_maps, pp=pp)
    return (rt_o, mv_o)



# revision 4
# speedup vs baseline: 1.7747x; 1.7747x over previous
"""GraphSAGE-max (3 layers + 2 heads) on 8 Trainium2 NeuronCores — v6.

SBUF-source transpose-mode dma_gather design. Tables are node-major in
SBUF (token = one node's 256 bf16 features = 512 B on partition
node%128, stripe node//128); the SDMA transpose crossbar returns
gathers FEATURE-major: out[p, c, i] = feature c*128+p of index i. So:

  - gathers run on the 16 SDMA engines (descriptor path, no HBM),
  - tree-max folds over the k axis of [128, 2, k, nodes] views (DVE 2x),
  - matmuls take the folded slabs directly (feature-major, bf16),
  - layer outputs transpose back (PE) into node-major stripe blocks
    which AllGather into the next layer's tables.

Tables hold one 3200-node column-half per core at a time (2 phases per
layer); each AllGather is split into two column-half collectives so the
second overlaps the next layer's first sweep. Calls pack several dst
tiles with a uniform k so one fold tree serves the whole call.
"""

import numpy as np
import ml_dtypes

import concourse.bass as bass
import concourse.bacc as bacc
import concourse.mybir as mybir
import concourse.tile as tile
from concourse.bass_utils import run_bass_kernel_spmd

N = 50000
E = 800000
F_IN = 128
H = 256
NCOR = 8
NLOC = N // NCOR             # 6250
PADH = 3200                  # padded nodes per (core, half): 25 stripes
HSTR = PADH // 128           # 25 stripes per (core, half)
TSTR = NCOR * HSTR + 1       # 201 table stripes (last = pad/zero rows)
PADIDX = NCOR * PADH         # -inf (L1) / 0 (L2,3) token
ZEROIDX = PADIDX + 1         # all-zero token (isolated nodes)
TILES = (NLOC + 127) // 128  # 49
PADN = TILES * 128           # 6272
ATILES = PADH // 128         # 25 tiles in column-half A
CHUNK = 4
NCH = (TILES + CHUNK - 1) // CHUNK
CALLCAP_K = 24               # uniform-k call capacity (in 128-idx units)
AGC = PADH // (CHUNK * 128)  # chunk whose stores complete blk half A
NEG = float(np.finfo(np.float32).min)

_LAST = {}


# ----------------------------------------------------------------------------
# host-side graph preprocessing
# ----------------------------------------------------------------------------

def _wrap_idx(ilist):
    ncor, num = ilist.shape
    assert num % 16 == 0
    cols = num // 16
    w = np.zeros((ncor, 16, cols), np.int16)
    i = np.arange(num)
    w[:, i % 16, i // 16] = ilist
    return np.tile(w, (1, 8, 1))


def _preprocess(edge_index):
    src = np.asarray(edge_index[0], np.int64)
    dst = np.asarray(edge_index[1], np.int64)
    deg = np.bincount(dst, minlength=N)

    order = np.argsort(-deg, kind="stable")
    ranks = np.arange(N)
    pos = ranks % NCOR
    core_of_rank = np.where((ranks // NCOR) % 2 == 0, pos, NCOR - 1 - pos)
    owner = np.empty(N, np.int64)
    owner[order] = core_of_rank

    # pass 1: deg-desc within core fixes the column-half split
    # (locals [0, PADH) = half A, the high-degree half)
    halfA_of_old = np.zeros(N, bool)
    for m in range(NCOR):
        nodes = np.where(owner == m)[0]
        key = np.argsort(-deg[nodes], kind="stable")
        halfA_of_old[nodes[key[:PADH]]] = True
    degA = np.bincount(dst[halfA_of_old[src]], minlength=N)

    # pass 2: within (core, half): (deg desc, degA desc)
    old_of_new = np.empty(N, np.int64)
    for m in range(NCOR):
        nodes = np.where(owner == m)[0]
        for h, base in ((True, 0), (False, PADH)):
            sub = nodes[halfA_of_old[nodes] == h]
            key = np.lexsort((-degA[sub], -deg[sub]))
            o = m * NLOC + base
            old_of_new[o:o + sub.size] = sub[key]
    new_of_old = np.empty(N, np.int64)
    new_of_old[old_of_new] = np.arange(N)

    nd = new_of_old[dst]
    src_new = new_of_old[src]
    src_loc = src_new % NLOC
    ph = (src_loc >= PADH).astype(np.int64)
    scol = (src_new // NLOC) * PADH + src_loc - ph * PADH
    gk = nd * 2 + ph
    eorder = np.argsort(gk, kind="stable")
    gk_s = gk[eorder]
    scol_s = scol[eorder]
    starts = np.searchsorted(gk_s, np.arange(2 * N))
    slot = np.arange(E) - starts[gk_s]
    cnt = np.bincount(gk, minlength=2 * N)
    dA = cnt[0::2]
    dB = cnt[1::2]

    def ktile(d):
        ks = np.zeros(TILES, np.int64)
        for m in range(NCOR):
            dm = d[m * NLOC:(m + 1) * NLOC]
            for t in range(TILES):
                blk = dm[t * 128:(t + 1) * 128]
                if blk.size:
                    ks[t] = max(ks[t], int(blk.max()))
        return ks
    kA = ktile(dA)
    kB = ktile(dB)

    def ell_of(phase, kmax):
        ell = np.full((NCOR, PADN, max(kmax, 1)), PADIDX, np.int16)
        sel = ph[eorder] == phase
        nd_e = nd[eorder][sel]
        ell[nd_e // NLOC, nd_e % NLOC, slot[sel]] = scol_s[sel].astype(np.int16)
        return ell

    ellA = ell_of(0, int(kA.max()))
    ellB = ell_of(1, int(kB.max()))

    iso = np.where(deg == 0)[0]
    if iso.size:
        ni = new_of_old[iso]
        ellA[ni // NLOC, ni % NLOC, 0] = ZEROIDX
        for t in np.unique((ni % NLOC) // 128):
            kA[t] = max(kA[t], 1)

    # ----- pack tiles into uniform-k calls ------------------------------
    # call: dict(ci, phase, col0, nidx, kc, segs=[(tile, slot, kreal, kstart)])
    calls = []

    def emit(phase, ks, groups):
        for tiles_grp in groups:
            queue = [(t, 0) for t in tiles_grp if ks[t] > 0]
            while queue:
                t0, ks0 = queue.pop(0)
                kc = min(int(ks[t0]) - ks0, CALLCAP_K)
                segs = [(t0, ks0, min(int(ks[t0]) - ks0, kc))]
                while queue:
                    t1, ks1 = queue[0]
                    k1 = min(int(ks[t1]) - ks1, CALLCAP_K)
                    nkc = max(kc, k1)
                    if nkc * (len(segs) + 1) > CALLCAP_K:
                        break
                    queue.pop(0)
                    kc = nkc
                    segs.append((t1, ks1, k1))
                cl = dict(phase=phase, kc=kc,
                          segs=[(t, s, kr, kst) for s, (t, kst, kr)
                                in enumerate(segs)])
                calls.append(cl)
                # continuations (only the first tile can be capped)
                t0, kst0, kr0 = segs[0]
                if kst0 + kr0 < int(ks[t0]):
                    queue.insert(0, (t0, kst0 + kr0))

    emit(0, kA, [sorted(range(TILES), key=lambda t: -kA[t])])
    emit(1, kB, [list(range(c * CHUNK, min((c + 1) * CHUNK, TILES)))
                 for c in range(NCH)])

    # idx streams: order [k][seg][node], pad to kc with PADIDX
    blocks = []
    col = 0
    for ci, cl in enumerate(calls):
        cl["ci"] = ci
        ell = ellA if cl["phase"] == 0 else ellB
        S = len(cl["segs"])
        kc = cl["kc"]
        flat = np.full((NCOR, kc, S, 128), PADIDX, np.int16)
        for (t, s, kreal, kstart) in cl["segs"]:
            blk = ell[:, t * 128:(t + 1) * 128, kstart:kstart + kreal]
            flat[:, :kreal, s, :] = blk.transpose(0, 2, 1)
        flat = flat.reshape(NCOR, kc * S * 128)
        cl["col0"] = col
        cl["nidx"] = flat.shape[1]
        blocks.append(_wrap_idx(flat))
        col += flat.shape[1] // 16
    idx_stream = np.concatenate(blocks, axis=2)

    last_call = {}
    for cl in calls:
        for (t, s, kreal, kstart) in cl["segs"]:
            last_call[(cl["phase"], t)] = cl["ci"]

    return dict(new_of_old=new_of_old, old_of_new=old_of_new,
                calls=calls, totcols=col, idx_stream=idx_stream,
                kA=kA, kB=kB, last_call=last_call)


# ----------------------------------------------------------------------------
# device program
# ----------------------------------------------------------------------------

def _build_program(calls, totcols, kA, kB, last_call):
    nc = bacc.Bacc("TRN2", target_bir_lowering=False, debug=False,
                   num_devices=NCOR, num_swdge_queues=4)
    f32, bf16, i16 = mybir.dt.float32, mybir.dt.bfloat16, mybir.dt.int16

    t_xtab = {h: nc.dram_tensor(f"xtab{h}", [128, TSTR * 128], bf16,
                                kind="ExternalInput") for h in (0, 1)}
    t_xTs = nc.dram_tensor("xTs", [128, PADN], bf16, kind="ExternalInput")
    t_idx = nc.dram_tensor("idx", [128, totcols], i16, kind="ExternalInput")
    wnames = ["Wl1", "Wr1", "Wl2", "Wr2", "Wla", "Wra", "Wlm", "Wrm"]
    wshapes = {"Wl1": (F_IN, H), "Wr1": (F_IN, H)}
    t_w = {w: nc.dram_tensor(w, list(wshapes.get(w, (H, H))), bf16,
                             kind="ExternalInput") for w in wnames}
    t_b = {b: nc.dram_tensor(b, [H, 1], f32, kind="ExternalInput")
           for b in ["bl1", "bl2", "bla", "blm"]}
    t_wh = {w: nc.dram_tensor(w, [H, 1], bf16, kind="ExternalInput")
            for w in ["Wa", "Wm"]}
    t_bh = {b: nc.dram_tensor(b, [1, 1], f32, kind="ExternalInput")
            for b in ["ba", "bm"]}
    t_out = nc.dram_tensor("out", [2, NLOC], f32, kind="ExternalOutput")

    calls_a = [c for c in calls if c["phase"] == 0]
    calls_b = [c for c in calls if c["phase"] == 1]
    calls_b_by_chunk = {c: [] for c in range(NCH)}
    for cl in calls_b:
        calls_b_by_chunk[cl["segs"][0][0] // CHUNK].append(cl)

    from concourse.masks import make_identity

    with tile.TileContext(nc) as tc:
        with tc.tile_pool(name="const", bufs=1) as cpool, \
             tc.tile_pool(name="table", bufs=1) as tpool, \
             tc.tile_pool(name="partp", bufs=1) as ppool, \
             tc.tile_pool(name="work", bufs=2) as wk, \
             tc.tile_pool(name="psY", bufs=2, space="PSUM") as psY, \
             tc.tile_pool(name="psT", bufs=2, space="PSUM") as psT, \
             tc.tile_pool(name="psH", bufs=2, space="PSUM") as psH, \
             tc.tile_pool(name="dram", bufs=1, space="DRAM") as dram:

            ident = cpool.tile([128, 128], f32, name="ident")
            make_identity(nc, ident[:])

            w_sb = {}
            for w in wnames:
                fi = wshapes.get(w, (H, H))[0]
                fh = fi // 128
                ws = cpool.tile([128, fh * H], bf16, name=f"sb_{w}")
                for h in range(fh):
                    nc.sync.dma_start(ws[:, h * H:(h + 1) * H],
                                      t_w[w][h * 128:(h + 1) * 128, :])
                w_sb[w] = ws
            b_sb = {}
            for b in t_b:
                bs = cpool.tile([128, 2], f32, name=f"sb_{b}")
                for h in range(2):
                    nc.sync.dma_start(bs[:, h:h + 1],
                                      t_b[b][h * 128:(h + 1) * 128, :])
                b_sb[b] = bs
            wh_sb = {}
            for w in t_wh:
                ws = cpool.tile([128, 2], bf16, name=f"sb_{w}")
                for h in range(2):
                    nc.sync.dma_start(ws[:, h:h + 1],
                                      t_wh[w][h * 128:(h + 1) * 128, :])
                wh_sb[w] = ws
            bh_sb = {}
            for b in t_bh:
                bs = cpool.tile([1, 1], f32, name=f"sb_{b}")
                nc.sync.dma_start(bs[:], t_bh[b][:])
                bh_sb[b] = bs

            idxsb = cpool.tile([128, totcols], i16, name="idxsb")
            nc.sync.dma_start(idxsb[:], t_idx[:])

            tabsb = tpool.tile([128, TSTR * 256], bf16, name="tabsb")
            parts = ppool.tile([128, 2 * PADN], bf16, name="parts")

            blk = {(l, h): dram.tile([128, HSTR * 256], bf16,
                                     name=f"blk{l}{h}")
                   for l in (1, 2) for h in (0, 1)}
            htab = {(l, h): dram.tile([NCOR * 128, HSTR * 256], bf16,
                                      name=f"h{l}tab{h}",
                                      addr_space="Shared")
                    for l in (1, 2) for h in (0, 1)}
            hself = {1: dram.tile([128, 2 * PADN], bf16, name="h1self"),
                     2: dram.tile([128, 2 * PADN], bf16, name="h2self")}

            def load_table(lidx, half):
                if lidx == 0:
                    nc.sync.dma_start(tabsb[:, :TSTR * 128],
                                      t_xtab[half][:])
                else:
                    tab = htab[(lidx, half)]
                    for sec in range(NCOR):
                        nc.sync.dma_start(
                            tabsb[:, sec * HSTR * 256:
                                  (sec + 1) * HSTR * 256],
                            tab[sec * 128:(sec + 1) * 128, :])
                    # pad + zero tokens live in the last stripe
                    nc.vector.memset(
                        tabsb[0:2, (TSTR - 1) * 256:TSTR * 256], 0.0)

            def do_call(cl, nsl, primary, done, tag):
                """One transpose dma_gather + a single uniform-k fold.
                done[t] = (gtile, free_off) with slab c of tile t at
                g[:, c*nidx + off : c*nidx + off + 128]."""
                nidx = cl["nidx"]
                kc = cl["kc"]
                g = wk.tile([128, 2 * CALLCAP_K * 128], bf16,
                            name=f"g_{tag}", tag="gbuf", bufs=2)
                nc.gpsimd.dma_gather(
                    out_ap=g[:, :nsl * nidx].rearrange(
                        "p (c n) -> p c n", c=nsl),
                    in_ap=tabsb[:] if nsl == 2 else tabsb[:, :TSTR * 128],
                    idxs_ap=idxsb[:, cl["col0"]:cl["col0"] + nidx // 16],
                    num_idxs=nidx, num_idxs_reg=nidx,
                    elem_size=128 * nsl, transpose=True,
                    sbuf_tokens_per_rank=128,
                    sbuf_free_dim_per_rank=256 * nsl,
                    sbuf_free_dim_pad_per_rank=0,
                    sbuf_byte_offset=0, single_packet=False)
                # fold over k: view [128, nsl, k, S*128]
                v = g[:, :nsl * nidx].rearrange("p (c k w) -> p c k w",
                                                c=nsl, k=kc)
                k = kc
                while k > 1:
                    if k % 2 == 1:
                        nc.vector.tensor_tensor(
                            out=v[:, :, 0, :], in0=v[:, :, 0, :],
                            in1=v[:, :, k - 1, :], op=mybir.AluOpType.max)
                        k -= 1
                        if k == 1:
                            break
                    half = k // 2
                    nc.vector.tensor_tensor(
                        out=v[:, :, 0:half, :], in0=v[:, :, 0:half, :],
                        in1=v[:, :, half:2 * half, :],
                        op=mybir.AluOpType.max)
                    k = half
                for (t, s, kreal, kstart) in cl["segs"]:
                    off = s * 128
                    if kstart == 0:
                        primary[t] = (g, nidx, off)
                    else:
                        pg, pnidx, poff = primary[t]
                        for c in range(nsl):
                            nc.vector.tensor_tensor(
                                out=pg[:, c * pnidx + poff:
                                       c * pnidx + poff + 128],
                                in0=pg[:, c * pnidx + poff:
                                       c * pnidx + poff + 128],
                                in1=g[:, c * nidx + off:
                                      c * nidx + off + 128],
                                op=mybir.AluOpType.max)
                for (t, s, kreal, kstart) in cl["segs"]:
                    if last_call[(cl["phase"], t)] == cl["ci"]:
                        done[t] = primary.pop(t)

            def layer(lidx):
                is_l1 = lidx == 0
                nsl = 1 if is_l1 else 2
                Wl, Wr, bl = [("Wl1", "Wr1", "bl1"), ("Wl2", "Wr2", "bl2"),
                              (None, None, None)][lidx]

                # ---- phase A ------------------------------------------
                load_table(lidx, 0)
                primary, doneA = {}, {}
                for cl in calls_a:
                    do_call(cl, nsl, primary, doneA, f"A{lidx}")
                    for t in sorted(doneA):
                        g, nidx, off = doneA.pop(t)
                        for c in range(nsl):
                            nc.vector.tensor_copy(
                                parts[:, c * PADN + t * 128:
                                      c * PADN + (t + 1) * 128],
                                g[:, c * nidx + off:c * nidx + off + 128])

                # ---- phase B + chunk loop -----------------------------
                load_table(lidx, 1)
                primary, doneB = {}, {}
                for c in range(NCH):
                    ntile = min(CHUNK, TILES - c * CHUNK)
                    cw = ntile * 128
                    c0 = c * CHUNK * 128
                    for cl in calls_b_by_chunk[c]:
                        do_call(cl, nsl, primary, doneB, f"B{lidx}")

                    aggT = wk.tile([128, 2 * 512], bf16,
                                   name=f"aggT{lidx}", tag="aggT", bufs=1)
                    for i in range(ntile):
                        t = c * CHUNK + i
                        haveA = kA[t] > 0
                        haveB = t in doneB
                        if haveB:
                            g, nidx, off = doneB.pop(t)
                        for j in range(nsl):
                            outap = aggT[:, j * 512 + i * 128:
                                         j * 512 + (i + 1) * 128]
                            inA = parts[:, j * PADN + t * 128:
                                        j * PADN + (t + 1) * 128]
                            inB = (g[:, j * nidx + off:
                                     j * nidx + off + 128]
                                   if haveB else None)
                            if haveA and haveB:
                                nc.vector.tensor_tensor(
                                    out=outap, in0=inA, in1=inB,
                                    op=mybir.AluOpType.max)
                            elif haveB:
                                nc.vector.tensor_copy(outap, inB)
                            elif haveA:
                                nc.vector.tensor_copy(outap, inA)
                            else:
                                nc.vector.memset(outap, 0.0)

                    live = min(cw, NLOC - c0)
                    selfC = wk.tile([128, 2 * 512], bf16,
                                    name=f"selfC{lidx}", tag="selfC")
                    if is_l1:
                        nc.sync.dma_start(selfC[:, :cw],
                                          t_xTs[:, c0:c0 + cw])
                    else:
                        hs = hself[lidx]
                        for j in range(2):
                            nc.sync.dma_start(
                                selfC[:, j * 512:j * 512 + cw],
                                hs[:, j * PADN + c0:j * PADN + c0 + cw])

                    if lidx < 2:
                        houtC = wk.tile([128, 2 * 512], f32,
                                        name=f"hout{lidx}", tag="houtC")
                        for hh in range(2):
                            psy = psY.tile([128, 512], f32,
                                           name=f"psy{lidx}", tag="psy")
                            nmm = 2 * nsl
                            im = 0
                            for j in range(nsl):
                                nc.tensor.matmul(
                                    psy[:, :cw],
                                    w_sb[Wl][:, j * H + hh * 128:
                                             j * H + (hh + 1) * 128],
                                    aggT[:, j * 512:j * 512 + cw],
                                    start=(im == 0), stop=(im == nmm - 1))
                                im += 1
                                nc.tensor.matmul(
                                    psy[:, :cw],
                                    w_sb[Wr][:, j * H + hh * 128:
                                             j * H + (hh + 1) * 128],
                                    selfC[:, j * 512:j * 512 + cw],
                                    start=(im == 0), stop=(im == nmm - 1))
                                im += 1
                            nc.scalar.activation(
                                houtC[:, hh * 512:hh * 512 + cw],
                                psy[:, :cw],
                                mybir.ActivationFunctionType.Relu,
                                bias=b_sb[bl][:, hh:hh + 1])
                        # feature-major self slabs (bf16) for next layer
                        hs_out = hself[lidx + 1]
                        h16 = wk.tile([128, 2 * 512], bf16,
                                      name=f"h16_{lidx}", tag="h16")
                        for j in range(2):
                            nc.vector.tensor_copy(
                                h16[:, j * 512:j * 512 + cw],
                                houtC[:, j * 512:j * 512 + cw])
                            nc.sync.dma_start(
                                hs_out[:, j * PADN + c0:j * PADN + c0 + cw],
                                h16[:, j * 512:j * 512 + cw])
                        # node-major stripe blocks for the AllGather
                        for i in range(ntile):
                            t = c * CHUNK + i
                            half = 1 if t >= ATILES else 0
                            stripe = t - ATILES * half
                            nt = wk.tile([128, 256], bf16,
                                         name=f"nt{lidx}", tag="ntile",
                                         bufs=3)
                            for hh in range(2):
                                tp = psT.tile([128, 128], f32,
                                              name=f"tp{lidx}", tag="tp")
                                nc.tensor.transpose(
                                    tp[:],
                                    houtC[:, hh * 512 + i * 128:
                                          hh * 512 + (i + 1) * 128],
                                    ident[:])
                                nc.vector.tensor_copy(
                                    nt[:, hh * 128:(hh + 1) * 128], tp[:])
                            nc.sync.dma_start(
                                blk[(lidx + 1, half)][:, stripe * 256:
                                                      (stripe + 1) * 256],
                                nt[:])
                        if c == AGC:
                            nc.gpsimd.collective_compute(
                                "AllGather", mybir.AluOpType.bypass,
                                replica_groups=[list(range(NCOR))],
                                ins=[blk[(lidx + 1, 0)].opt()],
                                outs=[htab[(lidx + 1, 0)].opt()])
                    else:
                        for bi, (Wlx, Wrx, blx, Whx, bhx) in enumerate(
                                [("Wla", "Wra", "bla", "Wa", "ba"),
                                 ("Wlm", "Wrm", "blm", "Wm", "bm")]):
                            brT = wk.tile([128, 2 * 512], bf16,
                                          name=f"brT{bi}", tag=f"brT{bi}",
                                          bufs=1)
                            for hh in range(2):
                                psy = psY.tile([128, 512], f32,
                                               name=f"psy3_{bi}", tag="psy")
                                im = 0
                                for j in range(2):
                                    nc.tensor.matmul(
                                        psy[:, :cw],
                                        w_sb[Wlx][:, j * H + hh * 128:
                                                  j * H + (hh + 1) * 128],
                                        aggT[:, j * 512:j * 512 + cw],
                                        start=(im == 0), stop=False)
                                    im += 1
                                    nc.tensor.matmul(
                                        psy[:, :cw],
                                        w_sb[Wrx][:, j * H + hh * 128:
                                                  j * H + (hh + 1) * 128],
                                        selfC[:, j * 512:j * 512 + cw],
                                        start=False, stop=(im == 3))
                                    im += 1
                                nc.scalar.activation(
                                    brT[:, hh * 512:hh * 512 + cw],
                                    psy[:, :cw],
                                    mybir.ActivationFunctionType.Relu,
                                    bias=b_sb[blx][:, hh:hh + 1])
                            psh = psH.tile([1, 512], f32, name=f"psh{bi}",
                                           tag="psh")
                            for hh in range(2):
                                nc.tensor.matmul(
                                    psh[:, :cw], wh_sb[Whx][:, hh:hh + 1],
                                    brT[:, hh * 512:hh * 512 + cw],
                                    start=(hh == 0), stop=(hh == 1))
                            outsb = wk.tile([1, 512], f32,
                                            name=f"out{bi}", tag="outsb")
                            nc.scalar.activation(
                                outsb[:, :cw], psh[:, :cw],
                                mybir.ActivationFunctionType.Identity,
                                bias=bh_sb[bhx][:])
                            nc.sync.dma_start(
                                t_out[bi:bi + 1, c0:c0 + live],
                                outsb[:, :live])

                if lidx < 2:
                    nc.gpsimd.collective_compute(
                        "AllGather", mybir.AluOpType.bypass,
                        replica_groups=[list(range(NCOR))],
                        ins=[blk[(lidx + 1, 1)].opt()],
                        outs=[htab[(lidx + 1, 1)].opt()])

            layer(0)
            layer(1)
            layer(2)

    nc.compile()
    return nc


# ----------------------------------------------------------------------------
# entry point
# ----------------------------------------------------------------------------

def kernel(x, edge_index, Wl1, bl1, Wr1, Wl2, bl2, Wr2,
           Wla, bla, Wra, Wa, ba, Wlm, blm, Wrm, Wm, bm):
    x = np.asarray(x, np.float32)
    pp = _preprocess(edge_index)
    old_of_new = pp["old_of_new"]

    xp = x[old_of_new].astype(ml_dtypes.bfloat16)   # [N, F] node-major
    # node-major tables (128-feature = 256 B tokens for L1):
    # token u of half h = row at (u%128, stripe u//128)
    xtabs = []
    for h in (0, 1):
        tab = np.zeros((128, TSTR, F_IN), ml_dtypes.bfloat16)
        rows = np.zeros((NCOR * PADH, F_IN), ml_dtypes.bfloat16)
        for m in range(NCOR):
            s = m * NLOC + h * PADH
            n = min(PADH, NLOC - h * PADH)
            rows[m * PADH:m * PADH + n] = xp[s:s + n]
        tab[:, :TSTR - 1, :] = rows.reshape(TSTR - 1, 128, F_IN)\
            .transpose(1, 0, 2)
        tab[0, TSTR - 1, :] = ml_dtypes.bfloat16(NEG)   # PADIDX token
        # ZEROIDX token (partition 1) already zero
        xtabs.append(np.ascontiguousarray(tab.reshape(128, TSTR * F_IN)))

    nc = _build_program(pp["calls"], pp["totcols"], pp["kA"], pp["kB"],
                        pp["last_call"])

    def f32(a):
        return np.ascontiguousarray(np.asarray(a, np.float32))

    def b16(a):
        return np.ascontiguousarray(
            np.asarray(a, np.float32).astype(ml_dtypes.bfloat16))

    xT16 = np.ascontiguousarray(
        x[old_of_new].T.astype(ml_dtypes.bfloat16))   # [F, N]

    in_maps = []
    for m in range(NCOR):
        xTs = np.zeros((128, PADN), ml_dtypes.bfloat16)
        xTs[:, :NLOC] = xT16[:, m * NLOC:(m + 1) * NLOC]
        in_maps.append({
            "xtab0": xtabs[0], "xtab1": xtabs[1], "xTs": xTs,
            "idx": np.ascontiguousarray(pp["idx_stream"][m]),
            "Wl1": b16(Wl1), "Wr1": b16(Wr1),
            "Wl2": b16(Wl2), "Wr2": b16(Wr2),
            "Wla": b16(Wla), "Wra": b16(Wra),
            "Wlm": b16(Wlm), "Wrm": b16(Wrm),
            "bl1": f32(bl1).reshape(H, 1), "bl2": f32(bl2).reshape(H, 1),
            "bla": f32(bla).reshape(H, 1), "blm": f32(blm).reshape(H, 1),
            "Wa": b16(Wa).reshape(H, 1), "Wm": b16(Wm).reshape(H, 1),
            "ba": f32(ba).reshape(1, 1), "bm": f32(bm).reshape(1, 1),
        })

    res = run_bass_kernel_spmd(nc, in_maps, core_ids=list(range(NCOR)))

    rt = np.empty(N, np.float32)
    mv = np.empty(N, np.float32)
    for m in range(NCOR):
        out = res.results[m]["out"]
        rt[m * NLOC:(m + 1) * NLOC] = out[0]
        mv[m * NLOC:(m + 1) * NLOC] = out[1]
    rt_o = np.empty(N, np.float32)
    mv_o = np.empty(N, np.float32)
    rt_o[old_of_new] = rt
    mv_o[old_of_new] = mv

    _LAST.update(nc=nc, in_maps=in_maps, pp=pp)
    return (rt_o, mv_o)
